# revision 10
# baseline (speedup 1.0000x reference)
"""Single-head attention on 8 TRN2 NeuronCores, batch-parallel (1 batch elem/core).

reference (per batch b):
  qp = q[b] @ w_q; kp = k[b] @ w_k; vp = v[b] @ w_v        # [S,F]@[F,DK] -> [S,DK]
  scores = qp @ kp.T / sqrt(DK)                            # [S,S]
  out[b] = softmax(scores, axis=-1) @ vp                   # [S,DK]

Shapes: B=8, S=2048, F=1024, DK=128. f32 in/out, bf16 compute, f32 accumulate.

v2 structure (per core), built for DMA/compute overlap:
  1. Q phase: stream q groups, PE-transpose (bf16, cast in SWDGE DMA),
     project to qpT [DK, S].
  2. K/V streaming: per group g (4 s-tiles), load k & v, transpose, project
     kpT chunk / vp tiles; compute scoresT+exp for ALL sq into expT_all
     (SBUF); PV-accumulate the FIRST 8 sq-tiles in 3 psum banks (3
     accumulators of width 130 share one bank; a dummy zeroing matmul
     opens each bank since start=True clears the whole bank's has_written).
  3. Tail: PV for the second 8 sq-tiles (everything already in SBUF),
     divides, output DMA.

The ones column appended to vp makes the PV matmul accumulate the softmax
denominator in acc[:, 128] for free; out = acc[:, :128] * (1/acc[:, 128]).
"""
import numpy as np

B, S, F, DK = 8, 2048, 1024, 128
P = 128
N_CORES = 8
GT = 4                 # s-tiles per group
NG = S // (P * GT)     # 4 groups per input
NF = F // P            # 8 f-chunks
NT = S // P            # 16 s-tiles
NC4 = 4                # sq chunks of 512
SOFTMAX_SCALE = 1.0 / float(np.sqrt(DK))
AW = 130               # accumulator width (128 out + 1 denom + 1 pad)

_COMPILED = {}


def _patch_ldw_opt():
    # bass_utils hardcodes --enable-ldw-opt=false; fast weight load (FWL)
    # halves LDWEIGHTS time for bf16, and this kernel is LDWEIGHTS-heavy.
    import concourse.bass_utils as bu

    if getattr(bu.run_command, "_ldw_patched", False):
        return
    orig = bu.run_command

    def run_command(cmd, *a, **kw):
        # ldw-opt=true rejects bass's standalone InstLdweights; keep false.
        return orig(cmd, *a, **kw)

    run_command._ldw_patched = True
    bu.run_command = run_command


def _build():
    import concourse.bass as bass
    import concourse.mybir as mybir
    from concourse import bacc
    from concourse.tile import TileContext
    from concourse.masks import make_identity

    _patch_ldw_opt()

    f32 = mybir.dt.float32
    bf16 = mybir.dt.bfloat16
    EXP = mybir.ActivationFunctionType.Exp
    CPY = mybir.ActivationFunctionType.Copy

    nc = bacc.Bacc("TRN2", target_bir_lowering=False, debug=False,
                   num_devices=N_CORES)
    q_ext = nc.dram_tensor("q", [S, F], f32, kind="ExternalInput").ap()
    k_ext = nc.dram_tensor("k", [S, F], f32, kind="ExternalInput").ap()
    v_ext = nc.dram_tensor("v", [S, F], f32, kind="ExternalInput").ap()
    wq_ext = nc.dram_tensor("w_q", [F, DK], f32, kind="ExternalInput").ap()
    wk_ext = nc.dram_tensor("w_k", [F, DK], f32, kind="ExternalInput").ap()
    wv_ext = nc.dram_tensor("w_v", [F, DK], f32, kind="ExternalInput").ap()
    out_ext = nc.dram_tensor("out", [S, DK], f32, kind="ExternalOutput").ap()

    q_view = q_ext.rearrange("(n p) f -> p n f", p=P)
    k_view = k_ext.rearrange("(n p) f -> p n f", p=P)
    v_view = v_ext.rearrange("(n p) f -> p n f", p=P)
    out_view = out_ext.rearrange("(n p) d -> p n d", p=P)

    with TileContext(nc) as tc:
        with (
            tc.tile_pool(name="const", bufs=1) as const,
            tc.tile_pool(name="persist", bufs=1) as persist,
            tc.tile_pool(name="stage", bufs=4) as stage,
            tc.tile_pool(name="xtp", bufs=2) as xtp,
            tc.tile_pool(name="outp", bufs=4) as outp,
            tc.tile_pool(name="small", bufs=4) as small,
            tc.tile_pool(name="tp_ps", bufs=2, space="PSUM") as tp_ps,
            tc.tile_pool(name="pj_ps", bufs=1, space="PSUM") as pj_ps,
            tc.tile_pool(name="sc_ps", bufs=2, space="PSUM") as sc_ps,
            tc.tile_pool(name="ac_ps", bufs=1, space="PSUM") as ac_ps,
        ):
            ident = const.tile([P, P], bf16)
            make_identity(nc, ident)
            zeros_l = const.tile([P, P], bf16)
            nc.gpsimd.memset(zeros_l[:, :], 0.0)
            zeros_r = const.tile([P, 3 * AW], bf16)
            nc.gpsimd.memset(zeros_r[:, :], 0.0)

            # ~5us of dummy matmuls at t=0: pre-warms the HAM clock gate while
            # the first q group loads (~8us), so the real PE stream starts at
            # 2.4 GHz instead of warming up ~20us in.
            for _w in range(28):
                wu = sc_ps.tile([P, 3 * AW], f32, tag="sc", name="warmup")
                nc.tensor.matmul(wu[:, :], zeros_l[:, :], zeros_r[:, :],
                                 start=True, stop=True,
                                 skip_group_check=True)

            def load_group_early(view, g, nm):
                x_nat = stage.tile([P, GT, F], bf16, tag="stage", name=nm)
                h = GT // 2
                for hh in range(2):
                    nc.gpsimd.dma_start(
                        out=x_nat[:, hh * h:(hh + 1) * h, :],
                        in_=view[:, GT * g + hh * h:GT * g + (hh + 1) * h, :])
                return x_nat

            # first q group load goes ahead of the (strided, slow) weight DMAs
            q_nat0 = load_group_early(q_view, 0, "q_nat")

            wq_sb = const.tile([P, NF, DK], bf16)
            nc.gpsimd.dma_start(out=wq_sb[:, :, :],
                                in_=wq_ext.rearrange("(c p) d -> p c d", p=P))
            wk_sb = const.tile([P, NF, DK], bf16)
            nc.gpsimd.dma_start(out=wk_sb[:, :, :],
                                in_=wk_ext.rearrange("(c p) d -> p c d", p=P))
            wv_sb = const.tile([P, NF, DK], bf16)
            nc.gpsimd.dma_start(out=wv_sb[:, :, :],
                                in_=wv_ext.rearrange("(c p) d -> p c d", p=P))

            qpT = persist.tile([P, S], bf16)         # [DK, S]
            kpT = persist.tile([P, S], bf16)         # [DK, S]
            vp1 = persist.tile([P, NT, AW], bf16)    # [sk, tile, DK+1(+pad)]
            nc.gpsimd.memset(vp1[:, :, 128:129], 1.0)
            expT_all = persist.tile([P, NT, S], bf16)  # [sk, sk-tile, sq]

            # 3 accumulator banks; each holds 3 slots of width AW
            accs = [
                ac_ps.tile([P, 3 * AW], f32, tag=f"acc{i}", name=f"acc{i}")
                for i in range(3)
            ]

            def acc_slot(j):
                t = accs[(j % 8) // 3]
                s = ((j % 8) % 3) * AW
                return t, s

            def open_acc_banks():
                # dummy start=True matmul zeroing each whole acc bank
                for a in accs:
                    nc.tensor.matmul(a[:, :], zeros_l[:, :], zeros_r[:, :],
                                     start=True, stop=True,
                                     skip_group_check=True)

            def load_group(view, g, nm):
                return load_group_early(view, g, nm)

            def transpose_group(x_nat, nm):
                # [P, GT, F] bf16 (s on parts) -> [P, NF, GT*P] (f on parts)
                xT = xtp.tile([P, NF, GT * P], bf16, tag="xT", name=nm)
                for cc in range(NF // 2):
                    tp = tp_ps.tile([P, 2, GT * P], bf16, tag="tp", name="tp")
                    for ci in range(2):
                        c = 2 * cc + ci
                        for t in range(GT):
                            nc.tensor.transpose(
                                tp[:, ci, t * P:(t + 1) * P],
                                x_nat[:, t, c * P:(c + 1) * P],
                                ident[:, :])
                    nc.vector.tensor_copy(xT[:, 2 * cc:2 * cc + 2, :], tp[:, :, :])
                return xT

            def proj_qk(xT, w_sb, dstT, g):
                pj = pj_ps.tile([P, GT * P], f32, tag="pj", name="pj")
                for c in range(NF):
                    nc.tensor.matmul(pj[:, :], w_sb[:, c, :], xT[:, c, :],
                                     start=(c == 0), stop=(c == NF - 1))
                nc.scalar.copy(dstT[:, GT * P * g:GT * P * (g + 1)], pj[:, :])

            def proj_v(xT, g):
                for t in range(GT):
                    vps = pj_ps.tile([P, DK], f32, tag="pj", name="vps")
                    for c in range(NF):
                        nc.tensor.matmul(vps[:, :],
                                         xT[:, c, t * P:(t + 1) * P],
                                         wv_sb[:, c, :],
                                         start=(c == 0), stop=(c == NF - 1))
                    nc.vector.tensor_copy(vp1[:, GT * g + t, 0:128], vps[:, :])

            def scores_exp(t):
                for c in range(NC4):
                    sc = sc_ps.tile([P, GT * P], f32, tag="sc", name="sc")
                    nc.tensor.matmul(sc[:, :],
                                     kpT[:, t * P:(t + 1) * P],
                                     qpT[:, GT * P * c:GT * P * (c + 1)],
                                     start=True, stop=True)
                    nc.scalar.activation(
                        expT_all[:, t, GT * P * c:GT * P * (c + 1)],
                        sc[:, :], EXP, scale=SOFTMAX_SCALE)

            def pv_step(j, t, is_last):
                a, s = acc_slot(j)
                nc.tensor.matmul(a[:, s:s + 129],
                                 expT_all[:, t, j * P:(j + 1) * P],
                                 vp1[:, t, 0:129],
                                 start=False, stop=is_last,
                                 skip_group_check=True)

            def finish(j):
                a, s = acc_slot(j)
                rinv = small.tile([P, 1], f32, tag="rinv", name="rinv")
                nc.vector.reciprocal(rinv[:, :], a[:, s + 128:s + 129])
                out_t = outp.tile([P, DK], f32, tag="out", name="out_t")
                nc.scalar.activation(out_t[:, :], a[:, s:s + 128], CPY,
                                     scale=rinv[:, :])
                nc.sync.dma_start(out=out_view[:, j, :], in_=out_t[:, :])

            # ---- Q phase ----
            for g in range(NG):
                x_nat = q_nat0 if g == 0 else load_group(q_view, g, "q_nat")
                xT = transpose_group(x_nat, "qT")
                proj_qk(xT, wq_sb, qpT, g)

            # ---- K/V streaming ----
            open_acc_banks()
            for g in range(NG):
                xTk = transpose_group(load_group(k_view, g, "k_nat"), "kT")
                proj_qk(xTk, wk_sb, kpT, g)
                xTv = transpose_group(load_group(v_view, g, "v_nat"), "vT")
                proj_v(xTv, g)
                for t in range(GT * g, GT * (g + 1)):
                    scores_exp(t)
                for j in range(8):
                    for t in range(GT * g, GT * (g + 1)):
                        pv_step(j, t, t == NT - 1)
            for j in range(8):
                finish(j)

            # ---- tail: second 8 sq-tiles ----
            open_acc_banks()
            for j in range(8, 16):
                for t in range(NT):
                    pv_step(j, t, t == NT - 1)
            for j in range(8, 16):
                finish(j)

    nc.compile()
    return nc


def get_nc():
    if "nc" not in _COMPILED:
        _COMPILED["nc"] = _build()
    return _COMPILED["nc"]


def kernel(q, k, v, w_q, w_k, w_v):
    from concourse.bass_utils import run_bass_kernel_spmd

    q = np.ascontiguousarray(np.asarray(q, dtype=np.float32))
    k = np.ascontiguousarray(np.asarray(k, dtype=np.float32))
    v = np.ascontiguousarray(np.asarray(v, dtype=np.float32))
    w_q = np.ascontiguousarray(np.asarray(w_q, dtype=np.float32))
    w_k = np.ascontiguousarray(np.asarray(w_k, dtype=np.float32))
    w_v = np.ascontiguousarray(np.asarray(w_v, dtype=np.float32))

    nc = get_nc()
    in_maps = [
        {"q": q[b], "k": k[b], "v": v[b], "w_q": w_q, "w_k": w_k, "w_v": w_v}
        for b in range(B)
    ]
    res = run_bass_kernel_spmd(nc, in_maps, core_ids=list(range(N_CORES)))
    out = np.stack([res.results[b]["out"] for b in range(B)], axis=0)
    return out.astype(np.float32)



# revision 11
# speedup vs baseline: 1.1211x; 1.1211x over previous
"""Single-head attention on 8 TRN2 NeuronCores, batch-parallel (1 batch elem/core).

reference (per batch b):
  qp = q[b] @ w_q; kp = k[b] @ w_k; vp = v[b] @ w_v        # [S,F]@[F,DK] -> [S,DK]
  scores = qp @ kp.T / sqrt(DK)                            # [S,S]
  out[b] = softmax(scores, axis=-1) @ vp                   # [S,DK]

Shapes: B=8, S=2048, F=1024, DK=128. f32 in/out, bf16 compute, f32 accumulate.

v2 structure (per core), built for DMA/compute overlap:
  1. Q phase: stream q groups, PE-transpose (bf16, cast in SWDGE DMA),
     project to qpT [DK, S].
  2. K/V streaming: per group g (4 s-tiles), load k & v, transpose, project
     kpT chunk / vp tiles; compute scoresT+exp for ALL sq into expT_all
     (SBUF); PV-accumulate the FIRST 8 sq-tiles in 3 psum banks (3
     accumulators of width 130 share one bank; a dummy zeroing matmul
     opens each bank since start=True clears the whole bank's has_written).
  3. Tail: PV for the second 8 sq-tiles (everything already in SBUF),
     divides, output DMA.

The ones column appended to vp makes the PV matmul accumulate the softmax
denominator in acc[:, 128] for free; out = acc[:, :128] * (1/acc[:, 128]).
"""
import numpy as np

B, S, F, DK = 8, 2048, 1024, 128
P = 128
N_CORES = 8
GT = 4                 # s-tiles per group
NG = S // (P * GT)     # 4 groups per input
NF = F // P            # 8 f-chunks
NT = S // P            # 16 s-tiles
NC4 = 4                # sq chunks of 512
SOFTMAX_SCALE = 1.0 / float(np.sqrt(DK))
AW = 130               # accumulator width (128 out + 1 denom + 1 pad)

_COMPILED = {}


def _patch_ldw_opt():
    # bass_utils hardcodes --enable-ldw-opt=false; fast weight load (FWL)
    # halves LDWEIGHTS time for bf16, and this kernel is LDWEIGHTS-heavy.
    import concourse.bass_utils as bu

    if getattr(bu.run_command, "_ldw_patched", False):
        return
    orig = bu.run_command

    def run_command(cmd, *a, **kw):
        # ldw-opt=true rejects bass's standalone InstLdweights; keep false.
        return orig(cmd, *a, **kw)

    run_command._ldw_patched = True
    bu.run_command = run_command


def _build():
    import concourse.bass as bass
    import concourse.mybir as mybir
    from concourse import bacc
    from concourse.tile import TileContext
    from concourse.masks import make_identity

    _patch_ldw_opt()

    f32 = mybir.dt.float32
    bf16 = mybir.dt.bfloat16
    EXP = mybir.ActivationFunctionType.Exp
    CPY = mybir.ActivationFunctionType.Copy

    nc = bacc.Bacc("TRN2", target_bir_lowering=False, debug=False,
                   num_devices=N_CORES)
    q_ext = nc.dram_tensor("q", [S, F], f32, kind="ExternalInput").ap()
    k_ext = nc.dram_tensor("k", [S, F], f32, kind="ExternalInput").ap()
    v_ext = nc.dram_tensor("v", [S, F], f32, kind="ExternalInput").ap()
    wq_ext = nc.dram_tensor("w_q", [F, DK], f32, kind="ExternalInput").ap()
    wk_ext = nc.dram_tensor("w_k", [F, DK], f32, kind="ExternalInput").ap()
    wv_ext = nc.dram_tensor("w_v", [F, DK], f32, kind="ExternalInput").ap()
    out_ext = nc.dram_tensor("out", [S, DK], f32, kind="ExternalOutput").ap()

    q_view = q_ext.rearrange("(n p) f -> p n f", p=P)
    k_view = k_ext.rearrange("(n p) f -> p n f", p=P)
    v_view = v_ext.rearrange("(n p) f -> p n f", p=P)
    out_view = out_ext.rearrange("(n p) d -> p n d", p=P)

    with TileContext(nc) as tc:
        with (
            tc.tile_pool(name="const", bufs=1) as const,
            tc.tile_pool(name="persist", bufs=1) as persist,
            tc.tile_pool(name="stage", bufs=4) as stage,
            tc.tile_pool(name="xtp", bufs=2) as xtp,
            tc.tile_pool(name="outp", bufs=4) as outp,
            tc.tile_pool(name="small", bufs=4) as small,
            tc.tile_pool(name="tp_ps", bufs=2, space="PSUM") as tp_ps,
            tc.tile_pool(name="pj_ps", bufs=1, space="PSUM") as pj_ps,
            tc.tile_pool(name="sc_ps", bufs=2, space="PSUM") as sc_ps,
            tc.tile_pool(name="ac_ps", bufs=1, space="PSUM") as ac_ps,
        ):
            ident = const.tile([P, P], bf16)
            make_identity(nc, ident)
            zeros_l = const.tile([P, P], bf16)
            nc.gpsimd.memset(zeros_l[:, :], 0.0)
            zeros_r = const.tile([P, 3 * AW], bf16)
            nc.gpsimd.memset(zeros_r[:, :], 0.0)

            # ~5us of dummy matmuls at t=0: pre-warms the HAM clock gate while
            # the first q group loads (~8us), so the real PE stream starts at
            # 2.4 GHz instead of warming up ~20us in.
            for _w in range(28):
                wu = sc_ps.tile([P, 3 * AW], f32, tag="sc", name="warmup")
                nc.tensor.matmul(wu[:, :], zeros_l[:, :], zeros_r[:, :],
                                 start=True, stop=True,
                                 skip_group_check=True)

            def load_group_early(view, g, nm):
                x_nat = stage.tile([P, GT, F], bf16, tag="stage", name=nm)
                h = GT // 2
                for hh in range(2):
                    nc.gpsimd.dma_start(
                        out=x_nat[:, hh * h:(hh + 1) * h, :],
                        in_=view[:, GT * g + hh * h:GT * g + (hh + 1) * h, :])
                return x_nat

            # first q group load goes ahead of the (strided, slow) weight DMAs
            q_nat0 = load_group_early(q_view, 0, "q_nat")

            wq_sb = const.tile([P, NF, DK], bf16)
            nc.gpsimd.dma_start(out=wq_sb[:, :, :],
                                in_=wq_ext.rearrange("(c p) d -> p c d", p=P))
            wk_sb = const.tile([P, NF, DK], bf16)
            nc.gpsimd.dma_start(out=wk_sb[:, :, :],
                                in_=wk_ext.rearrange("(c p) d -> p c d", p=P))
            wv_sb = const.tile([P, NF, DK], bf16)
            nc.gpsimd.dma_start(out=wv_sb[:, :, :],
                                in_=wv_ext.rearrange("(c p) d -> p c d", p=P))

            qpT = persist.tile([P, S], bf16)         # [DK, S]
            kpT = persist.tile([P, S], bf16)         # [DK, S]
            vp1 = persist.tile([P, NT, AW], bf16)    # [sk, tile, DK+1(+pad)]
            nc.gpsimd.memset(vp1[:, :, 128:129], 1.0)
            expT_all = persist.tile([P, NT, S], bf16)  # [sk, sk-tile, sq]

            # 3 accumulator banks; each holds 3 slots of width AW
            accs = [
                ac_ps.tile([P, 3 * AW], f32, tag=f"acc{i}", name=f"acc{i}")
                for i in range(3)
            ]

            def acc_slot(j):
                t = accs[(j % 8) // 3]
                s = ((j % 8) % 3) * AW
                return t, s

            def open_acc_banks():
                # dummy start=True matmul zeroing each whole acc bank
                for a in accs:
                    nc.tensor.matmul(a[:, :], zeros_l[:, :], zeros_r[:, :],
                                     start=True, stop=True,
                                     skip_group_check=True)

            def load_group(view, g, nm):
                return load_group_early(view, g, nm)

            def transpose_group(x_nat, nm):
                # [P, GT, F] bf16 (s on parts) -> [P, NF, GT*P] (f on parts)
                xT = xtp.tile([P, NF, GT * P], bf16, tag="xT", name=nm)
                for cc in range(NF // 2):
                    tp = tp_ps.tile([P, 2, GT * P], bf16, tag="tp", name="tp")
                    for ci in range(2):
                        c = 2 * cc + ci
                        for t in range(GT):
                            nc.tensor.transpose(
                                tp[:, ci, t * P:(t + 1) * P],
                                x_nat[:, t, c * P:(c + 1) * P],
                                ident[:, :])
                    nc.vector.tensor_copy(xT[:, 2 * cc:2 * cc + 2, :], tp[:, :, :])
                return xT

            def proj_qk(xT, w_sb, dstT, g):
                pj = pj_ps.tile([P, GT * P], f32, tag="pj", name="pj")
                for c in range(NF):
                    nc.tensor.matmul(pj[:, :], w_sb[:, c, :], xT[:, c, :],
                                     start=(c == 0), stop=(c == NF - 1))
                nc.scalar.copy(dstT[:, GT * P * g:GT * P * (g + 1)], pj[:, :])

            def proj_v(xT, g):
                for t in range(GT):
                    vps = pj_ps.tile([P, DK], f32, tag="pj", name="vps")
                    for c in range(NF):
                        nc.tensor.matmul(vps[:, :],
                                         xT[:, c, t * P:(t + 1) * P],
                                         wv_sb[:, c, :],
                                         start=(c == 0), stop=(c == NF - 1))
                    nc.vector.tensor_copy(vp1[:, GT * g + t, 0:128], vps[:, :])

            def scores_exp(t):
                for c in range(NC4):
                    sc = sc_ps.tile([P, GT * P], f32, tag="sc", name="sc")
                    nc.tensor.matmul(sc[:, :],
                                     kpT[:, t * P:(t + 1) * P],
                                     qpT[:, GT * P * c:GT * P * (c + 1)],
                                     start=True, stop=True)
                    nc.scalar.activation(
                        expT_all[:, t, GT * P * c:GT * P * (c + 1)],
                        sc[:, :], EXP, scale=SOFTMAX_SCALE)

            def pv_step(j, t, is_last):
                a, s = acc_slot(j)
                nc.tensor.matmul(a[:, s:s + 129],
                                 expT_all[:, t, j * P:(j + 1) * P],
                                 vp1[:, t, 0:129],
                                 start=False, stop=is_last,
                                 skip_group_check=True)

            def finish(j):
                a, s = acc_slot(j)
                rinv = small.tile([P, 1], f32, tag="rinv", name="rinv")
                nc.vector.reciprocal(rinv[:, :], a[:, s + 128:s + 129])
                out_t = outp.tile([P, DK], f32, tag="out", name="out_t")
                nc.scalar.activation(out_t[:, :], a[:, s:s + 128], CPY,
                                     scale=rinv[:, :])
                nc.sync.dma_start(out=out_view[:, j, :], in_=out_t[:, :])

            # ---- Q phase ----
            for g in range(NG):
                x_nat = q_nat0 if g == 0 else load_group(q_view, g, "q_nat")
                xT = transpose_group(x_nat, "qT")
                proj_qk(xT, wq_sb, qpT, g)

            # ---- K/V streaming ----
            open_acc_banks()
            for g in range(NG):
                xTk = transpose_group(load_group(k_view, g, "k_nat"), "kT")
                proj_qk(xTk, wk_sb, kpT, g)
                xTv = transpose_group(load_group(v_view, g, "v_nat"), "vT")
                proj_v(xTv, g)
                for t in range(GT * g, GT * (g + 1)):
                    scores_exp(t)
                for j in range(8):
                    for t in range(GT * g, GT * (g + 1)):
                        pv_step(j, t, t == NT - 1)
            for j in range(8):
                finish(j)

            # ---- tail: second 8 sq-tiles ----
            open_acc_banks()
            for j in range(8, 16):
                for t in range(NT):
                    pv_step(j, t, t == NT - 1)
            for j in range(8, 16):
                finish(j)

    _dedup_ldweights(nc, mybir)
    nc.compile()
    return nc


def _dedup_ldweights(nc, mybir):
    """Drop InstLdweights that reload the stationary already in the PE array
    (consecutive matmuls sharing one stationary, e.g. the 4 scores matmuls
    per kpT tile).  The paired InstMatmult has ldweights=False and simply
    reuses the loaded weights; ~36 weight reloads x ~131ns each."""
    import json

    def sig(i):
        d = json.loads(mybir.instruction_to_pretty_json_string(i))
        return json.dumps([d.get('ins'), d.get('is_transpose'),
                           d.get('perf_mode'), d.get('tile_position')],
                          sort_keys=True)

    for f in nc.m.functions:
        for blk in f.blocks:
            insts = list(blk.instructions)
            keep = []
            last = None
            removed = False
            for i in insts:
                tn = type(i).__name__
                if tn == 'InstLdweights':
                    s = sig(i)
                    if s == last and getattr(i, 'on_wait', None) in (None, []):
                        removed = True
                        continue
                    last = s
                elif tn != 'InstMatmult':
                    if getattr(i, 'engine', None) == mybir.EngineType.PE:
                        last = None
                keep.append(i)
            if removed:
                while len(blk.instructions):
                    blk.instructions.pop()
                for i in keep:
                    blk.instructions.append(i)


def get_nc():
    if "nc" not in _COMPILED:
        _COMPILED["nc"] = _build()
    return _COMPILED["nc"]


def kernel(q, k, v, w_q, w_k, w_v):
    from concourse.bass_utils import run_bass_kernel_spmd

    q = np.ascontiguousarray(np.asarray(q, dtype=np.float32))
    k = np.ascontiguousarray(np.asarray(k, dtype=np.float32))
    v = np.ascontiguousarray(np.asarray(v, dtype=np.float32))
    w_q = np.ascontiguousarray(np.asarray(w_q, dtype=np.float32))
    w_k = np.ascontiguousarray(np.asarray(w_k, dtype=np.float32))
    w_v = np.ascontiguousarray(np.asarray(w_v, dtype=np.float32))

    nc = get_nc()
    in_maps = [
        {"q": q[b], "k": k[b], "v": v[b], "w_q": w_q, "w_k": w_k, "w_v": w_v}
        for b in range(B)
    ]
    res = run_bass_kernel_spmd(nc, in_maps, core_ids=list(range(N_CORES)))
    out = np.stack([res.results[b]["out"] for b in range(B)], axis=0)
    return out.astype(np.float32)

